# revision 8
# baseline (speedup 1.0000x reference)
"""Trainium2 Bass kernel: receptive-field one-hot time encoder.

reference semantics:
    t_spike[n, b] = clip(int(scaling[n] * |x[b] - center[n]|), 0, T-1)
    out[t, b, n]  = 1.0 if t == t_spike[n, b] else 0.0       # [T, B, N] f32

Data-parallel over 8 cores: x and the output batch axis are sharded into 8
contiguous blocks of B_c = 16384; center/scaling are replicated.

Per-core layout: x viewed as [128 partitions, 128 cols]; all per-(b, n)
quantities live in [128, 2048] tiles with free index f*16+n, which makes each
output t-slice a single fully contiguous 1 MiB DRAM store ([T, B_c, N] with
b = p*128+f).

Pipeline (all DVE, broadcast 0-stride APs for the per-n vectors):
  per f-chunk c (4 chunks of 32 cols):
    diff = x - center       (tensor_tensor, broadcast APs)
    u    = |diff|           (ACT Abs)
    v    = u * scaling      (tensor_tensor)
    t=0  one-hot = is_lt(v, 1)   -- computable pre-floor: floor(v)==0 <=> v<1
    t=63 one-hot = is_ge(v, 63)  -- and the clip: floor(v)>=63 <=> v>=63
    floor(v) robust to convert rounding (sim truncates, HW rounds-to-nearest):
        r = f32(i32(v)); tsp = r - (r > v)
    ramp t=1..4 one-hots on this chunk   (keeps the DMA stream fed during prep)
  then t=5..62 full-width: one tensor_scalar is_equal + one 1 MiB DMA each.

The one-hot compares are exact: v is an exact f32 product when compared, tsp
is an exact small integer in f32, so outputs match the jax reference bit-for-
bit (relative error 0.0, verified on HW).
"""

import sys

if "/opt/trn_rl_repo" not in sys.path:
    sys.path.insert(0, "/opt/trn_rl_repo")

import numpy as np

import concourse.tile as tile
from concourse import bacc, mybir
from concourse.alu_op_type import AluOpType
from concourse.bass_utils import run_bass_kernel_spmd

B, N, T = 131072, 16, 64
NCORES = 8
BC = B // NCORES  # 16384 per core
P = 128
F = BC // P  # 128 x-columns per partition
FN = F * N  # 2048 free elems per output row

N_CHUNKS = 4
RAMP_K = 4
OUT_BUFS = 6

f32 = mybir.dt.float32
i32 = mybir.dt.int32

_cached = {}


def build_program():
    nc = bacc.Bacc(
        "TRN2", target_bir_lowering=False, debug=False, enable_asserts=False
    )
    # packed input: [:, :N]=center, [:, N:2N]=scaling, [:, 2N:]=x
    pk_d = nc.dram_tensor("packed", [P, 2 * N + F], f32, kind="ExternalInput").ap()
    out_d = nc.dram_tensor("out", [T, P, FN], f32, kind="ExternalOutput").ap()

    fc = F // N_CHUNKS
    wc = fc * N

    with tile.TileContext(nc) as tc:
        with (
            tc.tile_pool(name="prep", bufs=1) as prep,
            tc.tile_pool(name="outp", bufs=OUT_BUFS) as outp,
        ):
            pk_sb = prep.tile([P, 2 * N + F], f32)
            v = prep.tile([P, FN], f32)
            tsp = prep.tile([P, FN], f32)

            nc.sync.dma_start(pk_sb[:, : 2 * N + fc], pk_d[:, : 2 * N + fc])
            nc.sync.dma_start(pk_sb[:, 2 * N + fc :], pk_d[:, 2 * N + fc :])

            cen_b = (
                pk_sb[:, 0:N]
                .rearrange("p (f n) -> p f n", f=1)
                .broadcast_to([P, fc, N])
            )
            sca_b = (
                pk_sb[:, N : 2 * N]
                .rearrange("p (f n) -> p f n", f=1)
                .broadcast_to([P, fc, N])
            )

            for c in range(N_CHUNKS):
                xs = slice(2 * N + c * fc, 2 * N + (c + 1) * fc)
                ws = slice(c * wc, (c + 1) * wc)
                x_b = pk_sb[:, xs].broadcast_to([P, fc, N])
                diff = prep.tile([P, wc], f32, tag="diff")
                nc.vector.tensor_tensor(
                    diff[:].rearrange("p (f n) -> p f n", n=N),
                    x_b,
                    cen_b,
                    op=AluOpType.subtract,
                )
                u = prep.tile([P, wc], f32, tag="u")
                nc.scalar.activation(u[:], diff[:], mybir.ActivationFunctionType.Abs)
                nc.vector.tensor_tensor(
                    v[:, ws].rearrange("p (f n) -> p f n", n=N),
                    u[:].rearrange("p (f n) -> p f n", n=N),
                    sca_b,
                    op=AluOpType.mult,
                )
                o0 = outp.tile([P, wc], f32, tag="onehot_r")
                nc.vector.tensor_scalar(
                    o0[:], v[:, ws], 1.0, None, op0=AluOpType.is_lt
                )
                nc.sync.dma_start(out_d[0, :, ws], o0[:])
                o63 = outp.tile([P, wc], f32, tag="onehot_r")
                nc.vector.tensor_scalar(
                    o63[:], v[:, ws], float(T - 1), None, op0=AluOpType.is_ge
                )
                nc.sync.dma_start(out_d[T - 1, :, ws], o63[:])
                ri = prep.tile([P, wc], i32, tag="ri")
                rf = prep.tile([P, wc], f32, tag="rf")
                nc.vector.tensor_copy(ri[:], v[:, ws])
                nc.vector.tensor_copy(rf[:], ri[:])
                corr = prep.tile([P, wc], f32, tag="corr")
                nc.vector.tensor_tensor(
                    corr[:], rf[:], v[:, ws], op=AluOpType.is_gt
                )
                nc.vector.tensor_tensor(
                    tsp[:, ws], rf[:], corr[:], op=AluOpType.subtract
                )
                for t in range(1, 1 + RAMP_K):
                    o = outp.tile([P, wc], f32, tag="onehot_r")
                    nc.vector.tensor_scalar(
                        o[:], tsp[:, ws], float(t), None, op0=AluOpType.is_equal
                    )
                    nc.sync.dma_start(out_d[t, :, ws], o[:])

            for t in range(1 + RAMP_K, T - 1):
                o = outp.tile([P, FN], f32, tag="onehot")
                nc.vector.tensor_scalar(
                    o[:], tsp[:], float(t), None, op0=AluOpType.is_equal
                )
                nc.sync.dma_start(out_d[t], o[:])

    nc.compile()
    return nc


def _get_program():
    if "nc" not in _cached:
        _cached["nc"] = build_program()
    return _cached["nc"]


def kernel(x, center, scaling, time_steps):
    assert int(time_steps) == T
    x = np.ascontiguousarray(np.asarray(x, dtype=np.float32)).reshape(
        NCORES, P, F
    )
    cen = np.asarray(center, np.float32).reshape(N)
    sca = np.asarray(scaling, np.float32).reshape(N)
    packed = np.empty((NCORES, P, 2 * N + F), np.float32)
    packed[:, :, 0:N] = cen
    packed[:, :, N : 2 * N] = sca
    packed[:, :, 2 * N :] = x
    in_maps = [{"packed": packed[c]} for c in range(NCORES)]
    nc = _get_program()
    res = run_bass_kernel_spmd(nc, in_maps, core_ids=list(range(NCORES)))
    outs = [res.results[c]["out"].reshape(T, BC, N) for c in range(NCORES)]
    return np.concatenate(outs, axis=1)


# revision 10
# speedup vs baseline: 1.0112x; 1.0112x over previous
"""Trainium2 Bass kernel: receptive-field one-hot time encoder.

reference semantics:
    t_spike[n, b] = clip(int(scaling[n] * |x[b] - center[n]|), 0, T-1)
    out[t, b, n]  = 1.0 if t == t_spike[n, b] else 0.0       # [T, B, N] f32

Data-parallel over 8 cores: x and the output batch axis are sharded into 8
contiguous blocks of B_c = 16384; center/scaling are replicated.

Per-core layout: x viewed as [128 partitions, 128 cols]; all per-(b, n)
quantities live in [128, 2048] tiles with free index f*16+n, which makes each
output t-slice a single fully contiguous 1 MiB DRAM store ([T, B_c, N] with
b = p*128+f).

Pipeline (all DVE, broadcast 0-stride APs for the per-n vectors):
  per f-chunk c (4 chunks of 32 cols):
    diff = x - center       (tensor_tensor, broadcast APs)
    u    = |diff|           (ACT Abs)
    v    = u * scaling      (tensor_tensor)
    t=0  one-hot = is_lt(v, 1)   -- computable pre-floor: floor(v)==0 <=> v<1
    t=63 one-hot = is_ge(v, 63)  -- and the clip: floor(v)>=63 <=> v>=63
    floor(v) robust to convert rounding (sim truncates, HW rounds-to-nearest):
        r = f32(i32(v)); tsp = r - (r > v)
    ramp t=1..4 one-hots on this chunk   (keeps the DMA stream fed during prep)
  then t=5..62 full-width: one tensor_scalar is_equal + one 1 MiB DMA each.

The one-hot compares are exact: v is an exact f32 product when compared, tsp
is an exact small integer in f32, so outputs match the jax reference bit-for-
bit (relative error 0.0, verified on HW).
"""

import sys

if "/opt/trn_rl_repo" not in sys.path:
    sys.path.insert(0, "/opt/trn_rl_repo")

import numpy as np

import concourse.tile as tile
from concourse import bacc, mybir
from concourse.alu_op_type import AluOpType
from concourse.bass_utils import run_bass_kernel_spmd

B, N, T = 131072, 16, 64
NCORES = 8
BC = B // NCORES  # 16384 per core
P = 128
F = BC // P  # 128 x-columns per partition
FN = F * N  # 2048 free elems per output row

N_CHUNKS = 4
RAMP_K = 5
OUT_BUFS = 8

f32 = mybir.dt.float32
i32 = mybir.dt.int32

_cached = {}


def build_program():
    nc = bacc.Bacc(
        "TRN2", target_bir_lowering=False, debug=False, enable_asserts=False
    )
    # packed input: [:, :N]=center, [:, N:2N]=scaling, [:, 2N:]=x
    pk_d = nc.dram_tensor("packed", [P, 2 * N + F], f32, kind="ExternalInput").ap()
    out_d = nc.dram_tensor("out", [T, P, FN], f32, kind="ExternalOutput").ap()

    fc = F // N_CHUNKS
    wc = fc * N

    with tile.TileContext(nc) as tc:
        with (
            tc.tile_pool(name="prep", bufs=1) as prep,
            tc.tile_pool(name="outp", bufs=OUT_BUFS) as outp,
        ):
            pk_sb = prep.tile([P, 2 * N + F], f32)
            v = prep.tile([P, FN], f32)
            tsp = prep.tile([P, FN], f32)

            nc.sync.dma_start(pk_sb[:, : 2 * N + fc], pk_d[:, : 2 * N + fc])
            nc.sync.dma_start(pk_sb[:, 2 * N + fc :], pk_d[:, 2 * N + fc :])

            cen_b = (
                pk_sb[:, 0:N]
                .rearrange("p (f n) -> p f n", f=1)
                .broadcast_to([P, fc, N])
            )
            sca_b = (
                pk_sb[:, N : 2 * N]
                .rearrange("p (f n) -> p f n", f=1)
                .broadcast_to([P, fc, N])
            )

            for c in range(N_CHUNKS):
                xs = slice(2 * N + c * fc, 2 * N + (c + 1) * fc)
                ws = slice(c * wc, (c + 1) * wc)
                x_b = pk_sb[:, xs].broadcast_to([P, fc, N])
                diff = prep.tile([P, wc], f32, tag="diff")
                nc.vector.tensor_tensor(
                    diff[:].rearrange("p (f n) -> p f n", n=N),
                    x_b,
                    cen_b,
                    op=AluOpType.subtract,
                )
                u = prep.tile([P, wc], f32, tag="u")
                nc.scalar.activation(u[:], diff[:], mybir.ActivationFunctionType.Abs)
                nc.vector.tensor_tensor(
                    v[:, ws].rearrange("p (f n) -> p f n", n=N),
                    u[:].rearrange("p (f n) -> p f n", n=N),
                    sca_b,
                    op=AluOpType.mult,
                )
                o0 = outp.tile([P, wc], f32, tag="onehot_r")
                nc.vector.tensor_scalar(
                    o0[:], v[:, ws], 1.0, None, op0=AluOpType.is_lt
                )
                nc.sync.dma_start(out_d[0, :, ws], o0[:])
                o63 = outp.tile([P, wc], f32, tag="onehot_r")
                nc.vector.tensor_scalar(
                    o63[:], v[:, ws], float(T - 1), None, op0=AluOpType.is_ge
                )
                nc.sync.dma_start(out_d[T - 1, :, ws], o63[:])
                # converts routed off DVE (ACT + GPSIMD); the r-(r>v) fix
                # makes any convert rounding mode exact
                ri = prep.tile([P, wc], i32, tag="ri")
                rf = prep.tile([P, wc], f32, tag="rf")
                nc.scalar.activation(
                    ri[:], v[:, ws], mybir.ActivationFunctionType.Copy
                )
                nc.gpsimd.tensor_copy(rf[:], ri[:])
                corr = prep.tile([P, wc], f32, tag="corr")
                nc.vector.tensor_tensor(
                    corr[:], rf[:], v[:, ws], op=AluOpType.is_gt
                )
                nc.vector.tensor_tensor(
                    tsp[:, ws], rf[:], corr[:], op=AluOpType.subtract
                )
                for t in range(1, 1 + RAMP_K):
                    o = outp.tile([P, wc], f32, tag="onehot_r")
                    nc.vector.tensor_scalar(
                        o[:], tsp[:, ws], float(t), None, op0=AluOpType.is_equal
                    )
                    nc.sync.dma_start(out_d[t, :, ws], o[:])

            for t in range(1 + RAMP_K, T - 1):
                o = outp.tile([P, FN], f32, tag="onehot")
                nc.vector.tensor_scalar(
                    o[:], tsp[:], float(t), None, op0=AluOpType.is_equal
                )
                nc.sync.dma_start(out_d[t], o[:])

    nc.compile()
    return nc


def _get_program():
    if "nc" not in _cached:
        _cached["nc"] = build_program()
    return _cached["nc"]


def kernel(x, center, scaling, time_steps):
    assert int(time_steps) == T
    x = np.ascontiguousarray(np.asarray(x, dtype=np.float32)).reshape(
        NCORES, P, F
    )
    cen = np.asarray(center, np.float32).reshape(N)
    sca = np.asarray(scaling, np.float32).reshape(N)
    packed = np.empty((NCORES, P, 2 * N + F), np.float32)
    packed[:, :, 0:N] = cen
    packed[:, :, N : 2 * N] = sca
    packed[:, :, 2 * N :] = x
    in_maps = [{"packed": packed[c]} for c in range(NCORES)]
    nc = _get_program()
    res = run_bass_kernel_spmd(nc, in_maps, core_ids=list(range(NCORES)))
    outs = [res.results[c]["out"].reshape(T, BC, N) for c in range(NCORES)]
    return np.concatenate(outs, axis=1)


# revision 11
# speedup vs baseline: 1.0130x; 1.0017x over previous
"""Trainium2 Bass kernel: receptive-field one-hot time encoder.

reference semantics:
    t_spike[n, b] = clip(int(scaling[n] * |x[b] - center[n]|), 0, T-1)
    out[t, b, n]  = 1.0 if t == t_spike[n, b] else 0.0       # [T, B, N] f32

Data-parallel over 8 cores: x and the output batch axis are sharded into 8
contiguous blocks of B_c = 16384; center/scaling are replicated.

Per-core layout: x viewed as [128 partitions, 128 cols]; all per-(b, n)
quantities live in [128, 2048] tiles with free index f*16+n, which makes each
output t-slice a single fully contiguous 1 MiB DRAM store ([T, B_c, N] with
b = p*128+f).

Pipeline (all DVE, broadcast 0-stride APs for the per-n vectors):
  per f-chunk c (4 chunks of 32 cols):
    diff = x - center       (tensor_tensor, broadcast APs)
    u    = |diff|           (ACT Abs)
    v    = u * scaling      (tensor_tensor)
    t=0  one-hot = is_lt(v, 1)   -- computable pre-floor: floor(v)==0 <=> v<1
    t=63 one-hot = is_ge(v, 63)  -- and the clip: floor(v)>=63 <=> v>=63
    floor(v) robust to convert rounding (sim truncates, HW rounds-to-nearest):
        r = f32(i32(v)); tsp = r - (r > v)
    ramp t=1..4 one-hots on this chunk   (keeps the DMA stream fed during prep)
  then t=5..62 full-width: one tensor_scalar is_equal + one 1 MiB DMA each.

The one-hot compares are exact: v is an exact f32 product when compared, tsp
is an exact small integer in f32, so outputs match the jax reference bit-for-
bit (relative error 0.0, verified on HW).
"""

import sys

if "/opt/trn_rl_repo" not in sys.path:
    sys.path.insert(0, "/opt/trn_rl_repo")

import numpy as np

import concourse.tile as tile
from concourse import bacc, mybir
from concourse.alu_op_type import AluOpType
from concourse.bass_utils import run_bass_kernel_spmd

B, N, T = 131072, 16, 64
NCORES = 8
BC = B // NCORES  # 16384 per core
P = 128
F = BC // P  # 128 x-columns per partition
FN = F * N  # 2048 free elems per output row

N_CHUNKS = 4
RAMP_K = 5
OUT_BUFS = 8

f32 = mybir.dt.float32
i32 = mybir.dt.int32

_cached = {}


def build_program():
    nc = bacc.Bacc(
        "TRN2", target_bir_lowering=False, debug=False, enable_asserts=False
    )
    # packed input: [:, :N]=center, [:, N:2N]=scaling, [:, 2N:]=x
    pk_d = nc.dram_tensor("packed", [P, 2 * N + F], f32, kind="ExternalInput").ap()
    out_d = nc.dram_tensor("out", [T, P, FN], f32, kind="ExternalOutput").ap()

    fc = F // N_CHUNKS
    wc = fc * N

    with tile.TileContext(nc) as tc:
        with (
            tc.tile_pool(name="prep", bufs=1) as prep,
            tc.tile_pool(name="pp", bufs=2) as pp,
            tc.tile_pool(name="outp", bufs=OUT_BUFS) as outp,
        ):
            pk_sb = prep.tile([P, 2 * N + F], f32)
            v = prep.tile([P, FN], f32)
            tsp = prep.tile([P, FN], f32)

            nc.sync.dma_start(pk_sb[:, : 2 * N + fc], pk_d[:, : 2 * N + fc])
            nc.sync.dma_start(pk_sb[:, 2 * N + fc :], pk_d[:, 2 * N + fc :])

            cen_b = (
                pk_sb[:, 0:N]
                .rearrange("p (f n) -> p f n", f=1)
                .broadcast_to([P, fc, N])
            )
            sca_b = (
                pk_sb[:, N : 2 * N]
                .rearrange("p (f n) -> p f n", f=1)
                .broadcast_to([P, fc, N])
            )

            rfs = {}

            def chunk_head(c):
                xs = slice(2 * N + c * fc, 2 * N + (c + 1) * fc)
                ws = slice(c * wc, (c + 1) * wc)
                x_b = pk_sb[:, xs].broadcast_to([P, fc, N])
                diff = prep.tile([P, wc], f32, tag="diff")
                nc.vector.tensor_tensor(
                    diff[:].rearrange("p (f n) -> p f n", n=N),
                    x_b,
                    cen_b,
                    op=AluOpType.subtract,
                )
                u = prep.tile([P, wc], f32, tag="u")
                nc.scalar.activation(u[:], diff[:], mybir.ActivationFunctionType.Abs)
                nc.vector.tensor_tensor(
                    v[:, ws].rearrange("p (f n) -> p f n", n=N),
                    u[:].rearrange("p (f n) -> p f n", n=N),
                    sca_b,
                    op=AluOpType.mult,
                )
                o0 = outp.tile([P, wc], f32, tag="onehot_r")
                nc.vector.tensor_scalar(
                    o0[:], v[:, ws], 1.0, None, op0=AluOpType.is_lt
                )
                nc.sync.dma_start(out_d[0, :, ws], o0[:])
                o63 = outp.tile([P, wc], f32, tag="onehot_r")
                nc.vector.tensor_scalar(
                    o63[:], v[:, ws], float(T - 1), None, op0=AluOpType.is_ge
                )
                nc.sync.dma_start(out_d[T - 1, :, ws], o63[:])
                # converts routed off DVE (ACT + GPSIMD); the r-(r>v) fix
                # makes any convert rounding mode exact
                ri = pp.tile([P, wc], i32, tag="ri")
                rf = pp.tile([P, wc], f32, tag="rf")
                nc.scalar.activation(
                    ri[:], v[:, ws], mybir.ActivationFunctionType.Copy
                )
                nc.gpsimd.tensor_copy(rf[:], ri[:])
                rfs[c] = rf

            def chunk_tail(c):
                # floor-fix completion + ramped early t's, emitted one chunk
                # behind chunk_head so the ACT/GPSIMD convert latency of
                # chunk c hides behind chunk c+1's DVE prep
                ws = slice(c * wc, (c + 1) * wc)
                rf = rfs.pop(c)
                corr = prep.tile([P, wc], f32, tag="corr")
                nc.vector.tensor_tensor(
                    corr[:], rf[:], v[:, ws], op=AluOpType.is_gt
                )
                nc.vector.tensor_tensor(
                    tsp[:, ws], rf[:], corr[:], op=AluOpType.subtract
                )
                for t in range(1, 1 + RAMP_K):
                    o = outp.tile([P, wc], f32, tag="onehot_r")
                    nc.vector.tensor_scalar(
                        o[:], tsp[:, ws], float(t), None, op0=AluOpType.is_equal
                    )
                    nc.sync.dma_start(out_d[t, :, ws], o[:])

            for c in range(N_CHUNKS):
                chunk_head(c)
                if c >= 1:
                    chunk_tail(c - 1)
            chunk_tail(N_CHUNKS - 1)

            for t in range(1 + RAMP_K, T - 1):
                o = outp.tile([P, FN], f32, tag="onehot")
                nc.vector.tensor_scalar(
                    o[:], tsp[:], float(t), None, op0=AluOpType.is_equal
                )
                nc.sync.dma_start(out_d[t], o[:])

    nc.compile()
    return nc


def _get_program():
    if "nc" not in _cached:
        _cached["nc"] = build_program()
    return _cached["nc"]


def kernel(x, center, scaling, time_steps):
    assert int(time_steps) == T
    x = np.ascontiguousarray(np.asarray(x, dtype=np.float32)).reshape(
        NCORES, P, F
    )
    cen = np.asarray(center, np.float32).reshape(N)
    sca = np.asarray(scaling, np.float32).reshape(N)
    packed = np.empty((NCORES, P, 2 * N + F), np.float32)
    packed[:, :, 0:N] = cen
    packed[:, :, N : 2 * N] = sca
    packed[:, :, 2 * N :] = x
    in_maps = [{"packed": packed[c]} for c in range(NCORES)]
    nc = _get_program()
    res = run_bass_kernel_spmd(nc, in_maps, core_ids=list(range(NCORES)))
    outs = [res.results[c]["out"].reshape(T, BC, N) for c in range(NCORES)]
    return np.concatenate(outs, axis=1)


# revision 12
# speedup vs baseline: 1.0138x; 1.0008x over previous
"""Trainium2 Bass kernel: receptive-field one-hot time encoder.

reference semantics:
    t_spike[n, b] = clip(int(scaling[n] * |x[b] - center[n]|), 0, T-1)
    out[t, b, n]  = 1.0 if t == t_spike[n, b] else 0.0       # [T, B, N] f32

Data-parallel over 8 cores: x and the output batch axis are sharded into 8
contiguous blocks of B_c = 16384; center/scaling are replicated.

Per-core layout: x viewed as [128 partitions, 128 cols]; all per-(b, n)
quantities live in [128, 2048] tiles with free index f*16+n, which makes each
output t-slice a single fully contiguous 1 MiB DRAM store ([T, B_c, N] with
b = p*128+f).

Pipeline (all DVE, broadcast 0-stride APs for the per-n vectors):
  per f-chunk c (4 chunks of 32 cols):
    diff = x - center       (tensor_tensor, broadcast APs)
    u    = |diff|           (ACT Abs)
    v    = u * scaling      (tensor_tensor)
    t=0  one-hot = is_lt(v, 1)   -- computable pre-floor: floor(v)==0 <=> v<1
    t=63 one-hot = is_ge(v, 63)  -- and the clip: floor(v)>=63 <=> v>=63
    floor(v) robust to convert rounding (sim truncates, HW rounds-to-nearest):
        r = f32(i32(v)); tsp = r - (r > v)
    ramp t=1..4 one-hots on this chunk   (keeps the DMA stream fed during prep)
  then t=5..62 full-width: one tensor_scalar is_equal + one 1 MiB DMA each.

The one-hot compares are exact: v is an exact f32 product when compared, tsp
is an exact small integer in f32, so outputs match the jax reference bit-for-
bit (relative error 0.0, verified on HW).
"""

import sys

if "/opt/trn_rl_repo" not in sys.path:
    sys.path.insert(0, "/opt/trn_rl_repo")

import numpy as np

import concourse.tile as tile
from concourse import bacc, mybir
from concourse.alu_op_type import AluOpType
from concourse.bass_utils import run_bass_kernel_spmd

B, N, T = 131072, 16, 64
NCORES = 8
BC = B // NCORES  # 16384 per core
P = 128
F = BC // P  # 128 x-columns per partition
FN = F * N  # 2048 free elems per output row

N_CHUNKS = 4
RAMP_K = 6
OUT_BUFS = 8

f32 = mybir.dt.float32
i32 = mybir.dt.int32

_cached = {}


def build_program():
    nc = bacc.Bacc(
        "TRN2", target_bir_lowering=False, debug=False, enable_asserts=False
    )
    # packed input: [:, :N]=center, [:, N:2N]=scaling, [:, 2N:]=x
    pk_d = nc.dram_tensor("packed", [P, 2 * N + F], f32, kind="ExternalInput").ap()
    out_d = nc.dram_tensor("out", [T, P, FN], f32, kind="ExternalOutput").ap()

    fc = F // N_CHUNKS
    wc = fc * N

    with tile.TileContext(nc) as tc:
        with (
            tc.tile_pool(name="prep", bufs=1) as prep,
            tc.tile_pool(name="pp", bufs=2) as pp,
            tc.tile_pool(name="outp", bufs=OUT_BUFS) as outp,
        ):
            pk_sb = prep.tile([P, 2 * N + F], f32)
            v = prep.tile([P, FN], f32)
            tsp = prep.tile([P, FN], f32)

            nc.sync.dma_start(pk_sb[:, : 2 * N + fc], pk_d[:, : 2 * N + fc])
            nc.sync.dma_start(pk_sb[:, 2 * N + fc :], pk_d[:, 2 * N + fc :])

            cen_b = (
                pk_sb[:, 0:N]
                .rearrange("p (f n) -> p f n", f=1)
                .broadcast_to([P, fc, N])
            )
            sca_b = (
                pk_sb[:, N : 2 * N]
                .rearrange("p (f n) -> p f n", f=1)
                .broadcast_to([P, fc, N])
            )

            rfs = {}

            def chunk_head(c):
                xs = slice(2 * N + c * fc, 2 * N + (c + 1) * fc)
                ws = slice(c * wc, (c + 1) * wc)
                x_b = pk_sb[:, xs].broadcast_to([P, fc, N])
                diff = prep.tile([P, wc], f32, tag="diff")
                nc.vector.tensor_tensor(
                    diff[:].rearrange("p (f n) -> p f n", n=N),
                    x_b,
                    cen_b,
                    op=AluOpType.subtract,
                )
                u = prep.tile([P, wc], f32, tag="u")
                nc.scalar.activation(u[:], diff[:], mybir.ActivationFunctionType.Abs)
                nc.vector.tensor_tensor(
                    v[:, ws].rearrange("p (f n) -> p f n", n=N),
                    u[:].rearrange("p (f n) -> p f n", n=N),
                    sca_b,
                    op=AluOpType.mult,
                )
                o0 = outp.tile([P, wc], f32, tag="onehot_r")
                nc.vector.tensor_scalar(
                    o0[:], v[:, ws], 1.0, None, op0=AluOpType.is_lt
                )
                nc.sync.dma_start(out_d[0, :, ws], o0[:])
                o63 = outp.tile([P, wc], f32, tag="onehot_r")
                nc.vector.tensor_scalar(
                    o63[:], v[:, ws], float(T - 1), None, op0=AluOpType.is_ge
                )
                nc.sync.dma_start(out_d[T - 1, :, ws], o63[:])
                # converts routed off DVE (ACT + GPSIMD); the r-(r>v) fix
                # makes any convert rounding mode exact
                ri = pp.tile([P, wc], i32, tag="ri")
                rf = pp.tile([P, wc], f32, tag="rf")
                nc.scalar.activation(
                    ri[:], v[:, ws], mybir.ActivationFunctionType.Copy
                )
                nc.gpsimd.tensor_copy(rf[:], ri[:])
                rfs[c] = rf

            def chunk_tail(c):
                # floor-fix completion + ramped early t's, emitted one chunk
                # behind chunk_head so the ACT/GPSIMD convert latency of
                # chunk c hides behind chunk c+1's DVE prep
                ws = slice(c * wc, (c + 1) * wc)
                rf = rfs.pop(c)
                corr = prep.tile([P, wc], f32, tag="corr")
                nc.vector.tensor_tensor(
                    corr[:], rf[:], v[:, ws], op=AluOpType.is_gt
                )
                nc.vector.tensor_tensor(
                    tsp[:, ws], rf[:], corr[:], op=AluOpType.subtract
                )
                for t in range(1, 1 + RAMP_K):
                    o = outp.tile([P, wc], f32, tag="onehot_r")
                    nc.vector.tensor_scalar(
                        o[:], tsp[:, ws], float(t), None, op0=AluOpType.is_equal
                    )
                    nc.sync.dma_start(out_d[t, :, ws], o[:])

            for c in range(N_CHUNKS):
                chunk_head(c)
                if c >= 1:
                    chunk_tail(c - 1)
            chunk_tail(N_CHUNKS - 1)

            for t in range(1 + RAMP_K, T - 1):
                o = outp.tile([P, FN], f32, tag="onehot")
                nc.vector.tensor_scalar(
                    o[:], tsp[:], float(t), None, op0=AluOpType.is_equal
                )
                nc.sync.dma_start(out_d[t], o[:])

    nc.compile()
    return nc


def _get_program():
    if "nc" not in _cached:
        _cached["nc"] = build_program()
    return _cached["nc"]


def kernel(x, center, scaling, time_steps):
    assert int(time_steps) == T
    x = np.ascontiguousarray(np.asarray(x, dtype=np.float32)).reshape(
        NCORES, P, F
    )
    cen = np.asarray(center, np.float32).reshape(N)
    sca = np.asarray(scaling, np.float32).reshape(N)
    packed = np.empty((NCORES, P, 2 * N + F), np.float32)
    packed[:, :, 0:N] = cen
    packed[:, :, N : 2 * N] = sca
    packed[:, :, 2 * N :] = x
    in_maps = [{"packed": packed[c]} for c in range(NCORES)]
    nc = _get_program()
    res = run_bass_kernel_spmd(nc, in_maps, core_ids=list(range(NCORES)))
    outs = [res.results[c]["out"].reshape(T, BC, N) for c in range(NCORES)]
    return np.concatenate(outs, axis=1)
